# revision 54
# baseline (speedup 1.0000x reference)
"""AliasFreeActivation (upsample2x -> leaky_relu -> 31x31 depthwise sinc conv
-> downsample2x) as a Trainium2 Bass/Tile kernel, data-parallel over 8 cores.

Math (per [128,128] image; B*C = 512 images, 64 per core):
  out = Dy @ Conv_F(lrelu(Uy @ x @ Ux^T)) @ Dx^T
With F = sum_r g_r h_r^T (SVD of the 31x31 filter, effective rank 11):
  out = sum_r M_r @ act @ N_r^T
  M_r = Dy @ Toeplitz(g_r) [128,256],  N_r = Dx @ Toeplitz(h_r) [128,256]
  act = lrelu(Uy @ x @ Ux^T) [256,256]
All heavy work is dense matmuls on TensorE; downsample is folded into M/N.

Device dataflow per image (out[m,n] = sum_k lhsT[k,m] rhs[k,n]):
  S1a: tmpT[c,Y]    = sum_y x[y,c] UyT[y,Y]
  S1b: actT[X,Y]    = sum_c UxT[c,X] tmpT[c,Y]   (+ lrelu on evac)
  A:   W[Y,(r,j)]   = sum_X actT[X,Y] NT[X,(r,j)]
  B:   out[i,(m,j)] = sum_{r,Yc} MT[Yc,(r,i)] W[Yc,(r,m,j)]   (PSUM accum)
"""
import contextlib
import os

import numpy as np

import concourse.bass as bass
import concourse.mybir as mybir
import concourse.tile as tile
from concourse import bacc
from concourse.bass_utils import run_bass_kernel_spmd

H = 128
H2 = 256
KF = 31
LRELU_SLOPE = 0.01
RANK_ENV = os.environ.get("AFA_RANK")
RANK = int(RANK_ENV) if RANK_ENV else 11   # default; run() adapts to the filter
BAND_A = os.environ.get("AFA_BAND", "1") == "1"
GROUP = 4
# PSUM->SBUF evacuation engine assignment (v=DVE, s=ScalarE/ACT, p=Pool):
# [0] fp16-W evac, [1] fp8-W evac, [4] tmpT, [5] out-copy.
# NOTE: GPSIMD/Pool cannot read PSUM (verifier-enforced), so only v/s here.
def _evac():
    return os.environ.get("AFA_EVAC", "vsvs_v_s").replace("_", "")
N_CORES = 8
N_IMG = 64                      # images per core (512 / 8)
DT_MM = {
    "float32": mybir.dt.float32,
    "float32r": mybir.dt.float32r,
    "bfloat16": mybir.dt.bfloat16,
    "float16": mybir.dt.float16,
}[os.environ.get("AFA_DT", "float16")]  # matmul operand dtype


# ---------------- host-side constants ----------------

def _ac_matrix(out_n, in_n):
    scale = (in_n - 1) / (out_n - 1)
    c = np.arange(out_n, dtype=np.float64) * scale
    i0 = np.clip(np.floor(c).astype(np.int64), 0, in_n - 2)
    w = c - i0
    M = np.zeros((out_n, in_n), dtype=np.float64)
    M[np.arange(out_n), i0] = 1.0 - w
    M[np.arange(out_n), i0 + 1] = w
    return M


def _toeplitz_same(h, n):
    T = np.zeros((n, n), dtype=np.float64)
    for u in range(len(h)):
        d = u - len(h) // 2
        if d >= 0:
            idx = np.arange(0, n - d)
        else:
            idx = np.arange(-d, n)
        T[idx, idx + d] += h[u]
    return T


def _segs_of(rank):
    """Split the rank-stacked 128-col blocks into PSUM-bank segments <= 512."""
    segs = []
    r = 0
    while r < rank:
        nr = min(4, rank - r)
        segs.append((r, nr))
        r += nr
    return segs


def _shift_mat(n, d):
    S = np.zeros((n, n))
    idx = np.arange(0, n - d) if d >= 0 else np.arange(-d, n)
    S[idx, idx + d] = 1.0
    return S


def _weighted_rank(F, rank):
    """Rank-`rank` approx of F minimizing the end-to-end error under the
    signal model act ~ U x U^T with white x: err = ||B^1/2 (F-Fr) B^1/2||_F
    with B the Gram of the composed per-tap maps Z_u = D S_u U."""
    kf = F.shape[0]
    D = _ac_matrix(H, H2)
    Uu = _ac_matrix(H2, H)
    Zs = [D @ _shift_mat(H2, u - kf // 2) @ Uu for u in range(kf)]
    B = np.zeros((kf, kf))
    for u in range(kf):
        for v in range(u, kf):
            B[u, v] = B[v, u] = np.sum(Zs[u] * Zs[v])
    w, V = np.linalg.eigh(B)
    w = np.maximum(w, 1e-12)
    Bh = (V * np.sqrt(w)) @ V.T
    Bih = (V / np.sqrt(w)) @ V.T
    Gm = Bh @ F @ Bh
    U_, S_, Vt_ = np.linalg.svd(Gm)
    Fr = Bih @ (U_[:, :rank] * S_[:rank]) @ Vt_[:rank] @ Bih
    return Fr


FP8_ON = os.environ.get("AFA_FP8", "1") == "1"


def _sample_act(Uu, n=4):
    rng = np.random.default_rng(1234)
    X = rng.standard_normal((n, H, H))
    A = Uu @ X @ Uu.T
    return np.where(A >= 0, A, LRELU_SLOPE * A)


def _make_consts(filt, rank, seg_layout=True):
    """Build device constants. Ranks [0, k16) go to fp16 nt/mt in seg layout
    (nt columns (seg, j, r_local)-major for pass-A banded 2D APs; mt
    rank-major). When fp8 is active, the nr8 tail ranks go to nt8/mt8 in
    DoubleRow-interleaved fp8 layout, with per-rank scale balancing so act8,
    nt8, wg8 and mt8 all sit inside the fp8e4m3 normal range."""
    F = np.asarray(filt, dtype=np.float64)
    if os.environ.get("AFA_WSVD", "1") == "1" and rank < min(F.shape):
        F = _weighted_rank(F, rank)
    U, S, Vt = np.linalg.svd(F)
    D = _ac_matrix(H, H2)
    Uu = _ac_matrix(H2, H)
    uyt = np.ascontiguousarray(Uu.T).astype(np.float32)     # [128 y, 256 Y]

    nr8 = rank - 4 if (FP8_ON and seg_layout and 4 < rank <= 8) else 0
    k16 = rank - nr8

    Ms = [D @ _toeplitz_same(U[:, r] * np.sqrt(S[r]), H2) for r in range(rank)]
    Ns = [D @ _toeplitz_same(Vt[r, :] * np.sqrt(S[r]), H2) for r in range(rank)]

    nt = np.zeros((2, H, k16 * H), dtype=np.float32)
    mt = np.zeros((2, H, k16 * H), dtype=np.float32)
    segs = _segs_of(k16)
    for r in range(k16):
        if seg_layout:
            off = 0
            for (rs, nr) in segs:
                if rs <= r < rs + nr:
                    cols = off + np.arange(H) * nr + (r - rs)
                    break
                off += nr * H
        else:
            cols = np.arange(r * H, (r + 1) * H)
        for c in range(2):
            nt[c, :, cols] = Ns[r][:, c * H:(c + 1) * H].astype(np.float32)
            mt[c, :, r * H:(r + 1) * H] = Ms[r][:, c * H:(c + 1) * H].T.astype(np.float32)
    out = {"uyt": uyt, "uxt": uyt.copy(), "nt": nt, "mt": mt,
           "k16": k16, "nr8": nr8, "s_a": 1.0, "s_e": 1.0}
    if nr8 == 0:
        return out

    # ---- fp8 tail: per-rank scales ----
    act = _sample_act(Uu)
    actmax = np.abs(act).max() * 1.15
    s_a = 100.0 / actmax
    wmaxs = [np.abs(act @ Ns[r].T).max() * 1.3 for r in range(k16, rank)]
    mmaxs = [np.abs(Ms[r]).max() for r in range(k16, rank)]
    nmaxs = [np.abs(Ns[r]).max() for r in range(k16, rank)]
    # wg8_max_r * mt8_max_r = wmax_r * mmax_r is scale-invariant; split it
    # sqrt-balanced (bias wg8 up 2x), then choose global s_e to center nt8.
    bias = 2.0
    tmp = [np.sqrt(wmaxs[i] * mmaxs[i]) * bias / (s_a * wmaxs[i])
           for i in range(nr8)]           # = n_r * s_e per rank
    s_e = float(np.exp(np.mean(np.log([tmp[i] * nmaxs[i] for i in range(nr8)]))) / 100.0)
    n_rs = [tmp[i] / s_e for i in range(nr8)]
    m_rs = [1.0 / (s_a * n_rs[i] * s_e) for i in range(nr8)]

    np8 = mybir.dt.np(mybir.dt.float8e4)
    nt8 = np.zeros((H, 2, nr8 * H), dtype=np.float64)
    mt8 = np.zeros((H, 2, nr8 * H), dtype=np.float64)
    for i in range(nr8):
        r = k16 + i
        for c in range(2):
            # nt8[p, xc, j*nr8+i] = n_r * N_r[j, xc*128+p]
            nt8[:, c, i::nr8] = (n_rs[i] * Ns[r][:, c * H:(c + 1) * H]).T
            # mt8[p, yc, i*H+k] = m_r * M_r[k, yc*128+p]
            mt8[:, c, i * H:(i + 1) * H] = (m_rs[i] * Ms[r][:, c * H:(c + 1) * H]).T
    out["nt8"] = nt8.astype(np8).reshape(H, 2 * nr8 * H)
    out["mt8"] = mt8.astype(np8).reshape(H, 2 * nr8 * H)
    out["s_a"] = float(s_a)
    out["s_e"] = float(s_e)
    if os.environ.get("AFA_SAFOLD", "1") == "1":
        # Fold s_a into uyt so act comes out of S1/lrelu pre-scaled (lrelu is
        # positively homogeneous) and act8 becomes a PLAIN fp16->fp8 copy;
        # nt absorbs 1/s_a so the fp16-path W is unchanged. nt8 already
        # expects s_a-scaled act.
        out["uyt"] = (out["uyt"] * s_a).astype(np.float32)
        out["nt"] = (out["nt"] / s_a).astype(np.float32)
        out["s_a"] = 1.0
    return out


# ---------------- device program ----------------

def _build_tile_program(tc, outs, ins, *, n_img, rank, group, dt_mm, loop_reps=1,
                        k16=None, nr8=0, s_a=1.0, s_e=1.0):
    nc = tc.nc
    if nr8:
        x_d, uyt_d, uxt_d, nt_d, mt_d, nt8_d, mt8_d = ins
    else:
        x_d, uyt_d, uxt_d, nt_d, mt_d = ins
    out_d = outs[0]
    if k16 is None:
        k16 = rank
    RC = k16 * H
    G = group
    GW = G * H
    assert n_img % G == 0
    f32 = mybir.dt.float32
    fp8 = mybir.dt.float8e4
    DR = mybir.MatmulPerfMode.DoubleRow

    EVAC = _evac()
    segs = _segs_of(k16)
    n_units = 2 * k16 + nr8      # pass-B matmul count per group

    def _copy(eng, dst, src, scale=None):
        if eng == "v":
            if scale is None:
                nc.vector.tensor_copy(dst, src)
            else:
                nc.vector.tensor_scalar_mul(dst, src, scale)
        elif eng == "s":
            if scale is None:
                nc.scalar.activation(dst, src, mybir.ActivationFunctionType.Copy)
            else:
                nc.scalar.activation(dst, src, mybir.ActivationFunctionType.Copy,
                                     scale=scale)
        else:
            nc.gpsimd.tensor_copy(dst, src)

    ctx = contextlib.ExitStack()
    with ctx:
        const_pool = ctx.enter_context(tc.tile_pool(name="consts", bufs=1))
        x_pool = ctx.enter_context(tc.tile_pool(name="x", bufs=2))
        tmp_pool = ctx.enter_context(tc.tile_pool(
            name="tmp", bufs=int(os.environ.get("AFA_TMPB", "3"))))
        act_pool = ctx.enter_context(tc.tile_pool(
            name="act", bufs=int(os.environ.get("AFA_ACTB", "6"))))
        w_pool = ctx.enter_context(tc.tile_pool(
            name="w", bufs=int(os.environ.get("AFA_WB", "2"))))
        osb_pool = ctx.enter_context(tc.tile_pool(name="osb", bufs=2))
        # shared small-PSUM pool (tmpT pairs + act tiles), one bank per buf
        ps_s = ctx.enter_context(tc.tile_pool(
            name="ps_s", bufs=int(os.environ.get("AFA_PSS", "3")), space="PSUM"))
        ps_w = ctx.enter_context(tc.tile_pool(
            name="ps_w", bufs=int(os.environ.get("AFA_PSW", "2")), space="PSUM"))
        ps_out = ctx.enter_context(tc.tile_pool(name="ps_out", bufs=1, space="PSUM"))

        uyt_sb = const_pool.tile([H, H2], dt_mm, tag="uyt")
        nc.sync.dma_start(uyt_sb[:], uyt_d[:])
        uxt_sb = const_pool.tile([H, H2], dt_mm, tag="uxt")
        nc.sync.dma_start(uxt_sb[:], uxt_d[:])
        nt_sb = []
        mt_sb = []
        for c in range(2):
            t = const_pool.tile([H, RC], dt_mm, tag=f"nt{c}", name=f"nt{c}_sb")
            nc.sync.dma_start(t[:], nt_d[c])
            nt_sb.append(t)
            t = const_pool.tile([H, RC], dt_mm, tag=f"mt{c}", name=f"mt{c}_sb")
            nc.sync.dma_start(t[:], mt_d[c])
            mt_sb.append(t)
        if nr8:
            nt8_sb = const_pool.tile([H, 2 * nr8 * H], fp8, tag="nt8")
            nc.sync.dma_start(nt8_sb[:], nt8_d[:])
            mt8_sb = const_pool.tile([H, 2 * nr8 * H], fp8, tag="mt8")
            nc.sync.dma_start(mt8_sb[:], mt8_d[:])

        def _fetch_x(g):
            x_sb = x_pool.tile([H, GW], dt_mm, tag="x", name=f"x_{g}")
            xg = x_d[g * G:(g + 1) * G].rearrange("g h w -> h g w")
            nc.sync.dma_start(x_sb[:].rearrange("h (g w) -> h g w", g=G), xg)
            return x_sb

        def _emit_all_groups():
            ngroups = n_img // G
            pending = None
            x_next = _fetch_x(0)
            for g in range(ngroups):
                x_cur = x_next
                x_next = _fetch_x(g + 1) if g + 1 < ngroups else None
                pending = _emit_group(g, pending, x_cur)
            if pending is not None:
                for ci in range(3):
                    pending(ci, chunks=3)

        def _emit_group(g, pending_b, x_sb):

            wg_sb = w_pool.tile([H, 2 * k16 * GW], dt_mm, tag="wg",
                                name=f"wg_{g}")
            wgv = wg_sb[:].rearrange("p (c r g w) -> p c r g w", c=2, r=k16, g=G)
            wg8_sb = None
            if nr8:
                wg8_sb = w_pool.tile([H, 2 * nr8 * GW], fp8, tag="wg8",
                                     name=f"wg8_{g}")

            # phases 1+2 interleaved by half so ps_s slots for S1a pairs are
            # recycled from fast tmpT copies, not slow lrelus: the alloc
            # sequence tp0,ap0,ap1,tp1,ap2,ap3 with 3 bufs makes each tp wait
            # on the previous tp's DVE copy instead of an ACT lrelu.
            act_sbs = []
            act8_sbs = []
            for half in range(G // 2):
                tmpT_ps = ps_s.tile([H, 2 * H2], f32, tag="sp",
                                    name=f"tp_{g}_{half}")
                for u in range(2):
                    m = half * 2 + u
                    nc.tensor.matmul(tmpT_ps[:, u * H2:(u + 1) * H2],
                                     x_sb[:, m * H:(m + 1) * H], uyt_sb[:],
                                     start=True, stop=True)
                t_sb = tmp_pool.tile([H, 2 * H2], dt_mm, tag="tmpT")
                _copy(EVAC[4], t_sb[:], tmpT_ps[:])
                for u in range(2):
                    m = half * 2 + u
                    act_ps = ps_s.tile([H, 2 * H2], f32, tag="sp",
                                       name=f"ap_{g}_{m}")
                    tw = t_sb[:, u * H2:(u + 1) * H2]
                    for xc in range(2):
                        nc.tensor.matmul(act_ps[:, xc * H2:(xc + 1) * H2],
                                         uxt_sb[:, xc * H:(xc + 1) * H], tw,
                                         start=True, stop=True)
                    act_sb = act_pool.tile([H, 2 * H2], dt_mm, tag="act")
                    nc.scalar.activation(act_sb[:], act_ps[:],
                                         mybir.ActivationFunctionType.Lrelu,
                                         alpha=LRELU_SLOPE)
                    act_sbs.append(act_sb)
                    if nr8:
                        act8_sb = act_pool.tile([H, 2 * H2], fp8, tag="act8")
                        a8e = os.environ.get("AFA_ACT8ENG", "v")
                        if s_a == 1.0:
                            _copy(a8e, act8_sb[:], act_sb[:])
                        elif a8e == "p":
                            nc.gpsimd.tensor_scalar_mul(act8_sb[:], act_sb[:], s_a)
                        elif a8e == "v":
                            nc.vector.tensor_scalar_mul(act8_sb[:], act_sb[:], s_a)
                        else:
                            nc.scalar.activation(act8_sb[:], act_sb[:],
                                                 mybir.ActivationFunctionType.Copy,
                                                 scale=s_a)
                        act8_sbs.append(act8_sb)

            # phase 3: pass A per image, with the previous group's pass-B
            # matmuls interleaved between images so the evac engines always
            # have PE work to hide behind.
            # fp16 part: nt/W_ps seg columns are (j, r_local)-major, so the
            # Toeplitz j-band of each X-chunk is a CONTIGUOUS column window:
            # X-chunk0 only reaches j<=71, chunk1 only j>=56; j in [56,72)
            # accumulates (has_written set by mm1), the rest first-write.
            # fp8 part: one DoubleRow matmul per yc contracts both X-chunks.
            for m in range(G):
                if pending_b is not None and m > 0:
                    pending_b(m - 1, chunks=3)
                act_sb = act_sbs[m]
                soff = 0
                for si, (rs, nr) in enumerate(segs):
                    sw = nr * H
                    w_ps = ps_w.tile([H, 1024], f32, tag="wps",
                                     name=f"wps_{g}_{m}_{si}")
                    jwin = ((0, 72), (56, H)) if BAND_A else ((0, H), (0, H))
                    for yc in range(2):
                        for xc in range(2):
                            j0, j1 = jwin[xc]
                            nc.tensor.matmul(
                                w_ps[:, yc * 512 + j0 * nr:yc * 512 + j1 * nr],
                                act_sb[:, xc * H2 + yc * H: xc * H2 + (yc + 1) * H],
                                nt_sb[xc][:, soff + j0 * nr:soff + j1 * nr],
                                start=(xc == 0), stop=(xc == 1),
                                skip_group_check=BAND_A)
                    # evac: seg cols (yc-bank, j, r_local) -> wg (yc, r, m, j)
                    eng = EVAC[(0 if nr8 else si % 2 * 2) % 4]
                    if sw == 512:
                        src = w_ps[:].rearrange("p (c j r) -> p c r j", c=2, r=nr)
                        _copy(eng, wgv[:, :, rs:rs + nr, m], src)
                    else:
                        for yc in range(2):
                            src = w_ps[:, yc * 512:yc * 512 + sw].rearrange(
                                "p (j r) -> p r j", r=nr)
                            _copy(eng, wgv[:, yc, rs:rs + nr, m], src)
                    soff += sw
                if nr8:
                    act8 = act8_sbs[m][:].rearrange("p (x y) -> p x y", x=2)
                    nt8v = nt8_sb[:].rearrange("p (x c) -> p x c", x=2)
                    w_ps8 = ps_w.tile([H, 1024], f32, tag="wps",
                                      name=f"wps8_{g}_{m}")
                    for yc in range(2):
                        nc.tensor.matmul(
                            w_ps8[:, yc * 512:(yc + 1) * 512],
                            act8[:, :, yc * H:(yc + 1) * H],
                            nt8v,
                            start=True, stop=True, perf_mode=DR)
                    src = w_ps8[:].rearrange("p (c j r) -> p c r j", c=2, r=nr8)
                    dst = wg8_sb[:].rearrange(
                        "p (c r g w) -> p c r g w", c=2, r=nr8, g=G)[:, :, :, m]
                    _copy(EVAC[1], dst, src, scale=s_e)

            # pass B emitted in `chunks` slices; slice ci==chunks-1 finishes
            # the accumulation, evacuates and DMAs out
            state = {"out_ps": None, "nmm": 0}
            units = [("16", yc, r) for yc in range(2) for r in range(k16)]
            units += [("8", 0, rl) for rl in range(nr8)]

            def _pass_b(ci, chunks=G):
                if state["out_ps"] is None:
                    state["out_ps"] = ps_out.tile([H, GW], f32, tag="ops",
                                                  name=f"ops_{g}")
                out_ps = state["out_ps"]
                n0 = (ci * n_units) // chunks
                n1 = ((ci + 1) * n_units) // chunks
                for kind, yc, r in units[n0:n1]:
                    state["nmm"] += 1
                    if kind == "16":
                        nc.tensor.matmul(
                            out_ps[:],
                            mt_sb[yc][:, r * H:(r + 1) * H],
                            wgv[:, yc, r],
                            start=(state["nmm"] == 1),
                            stop=(state["nmm"] == n_units),
                            skip_group_check=True)
                    else:
                        mt8v = mt8_sb[:].rearrange("p (c ri) -> p c ri", c=2)
                        wg8v = wg8_sb[:].rearrange(
                            "p (c r gw) -> p c r gw", c=2, r=nr8)
                        nc.tensor.matmul(
                            out_ps[:],
                            mt8v[:, :, r * H:(r + 1) * H],
                            wg8v[:, :, r],
                            start=(state["nmm"] == 1),
                            stop=(state["nmm"] == n_units),
                            perf_mode=DR, skip_group_check=True)
                if ci == chunks - 1:
                    og = out_d[g * G:(g + 1) * G].rearrange("g h w -> h g w")
                    out_sb = osb_pool.tile([H, GW], f32, tag="osb")
                    _copy(EVAC[5], out_sb[:], out_ps[:])
                    nc.sync.dma_start(
                        og, out_sb[:].rearrange("h (g w) -> h g w", g=G))

            return _pass_b

        if loop_reps > 1:
            with tc.For_i(0, loop_reps, 1):
                _emit_all_groups()
        else:
            _emit_all_groups()


_NC_CACHE = {}


def _build_nc(n_img=N_IMG, rank=RANK, group=GROUP, dt_mm=DT_MM, loop_reps=1,
              k16=None, nr8=0, s_a=1.0, s_e=1.0):
    if k16 is None:
        k16 = rank
    key = (n_img, rank, group, dt_mm, loop_reps, k16, nr8,
           round(s_a, 9), round(s_e, 12), _evac(),
           os.environ.get("AFA_ACT8ENG", "v"), os.environ.get("AFA_PSW", ""),
           os.environ.get("AFA_PSS", ""))
    if key in _NC_CACHE:
        return _NC_CACHE[key]
    nc = bacc.Bacc("TRN2", target_bir_lowering=False, debug=False)
    f32 = mybir.dt.float32
    fp8 = mybir.dt.float8e4
    x_d = nc.dram_tensor("x", [n_img, H, H], dt_mm, kind="ExternalInput").ap()
    uyt_d = nc.dram_tensor("uyt", [H, H2], dt_mm, kind="ExternalInput").ap()
    uxt_d = nc.dram_tensor("uxt", [H, H2], dt_mm, kind="ExternalInput").ap()
    nt_d = nc.dram_tensor("nt", [2, H, k16 * H], dt_mm, kind="ExternalInput").ap()
    mt_d = nc.dram_tensor("mt", [2, H, k16 * H], dt_mm, kind="ExternalInput").ap()
    ins = [x_d, uyt_d, uxt_d, nt_d, mt_d]
    if nr8:
        nt8_d = nc.dram_tensor("nt8", [H, 2 * nr8 * H], fp8,
                               kind="ExternalInput").ap()
        mt8_d = nc.dram_tensor("mt8", [H, 2 * nr8 * H], fp8,
                               kind="ExternalInput").ap()
        ins += [nt8_d, mt8_d]
    out_d = nc.dram_tensor("out", [n_img, H, H], f32, kind="ExternalOutput").ap()
    with tile.TileContext(nc) as tc:
        _build_tile_program(tc, [out_d], ins,
                            n_img=n_img, rank=rank, group=group, dt_mm=dt_mm,
                            loop_reps=loop_reps, k16=k16, nr8=nr8,
                            s_a=s_a, s_e=s_e)
    nc.compile()
    _NC_CACHE[key] = nc
    return nc


def _pick_rank(filt):
    """Smallest rank whose weighted-truncation error estimate fits the
    error budget (harness gate 2e-2; leave room for fp16/fp8 quantization).
    For the reference's sinc filter this lands on 8."""
    if RANK_ENV:
        return int(RANK_ENV)
    F = np.asarray(filt, np.float64)
    if os.environ.get("AFA_WSVD", "1") == "1":
        kf = F.shape[0]
        D = _ac_matrix(H, H2)
        Uu = _ac_matrix(H2, H)
        Zs = [D @ _shift_mat(H2, u - kf // 2) @ Uu for u in range(kf)]
        B = np.zeros((kf, kf))
        for u in range(kf):
            for v in range(u, kf):
                B[u, v] = B[v, u] = np.sum(Zs[u] * Zs[v])
        w, V = np.linalg.eigh(B)
        Bh = (V * np.sqrt(np.maximum(w, 1e-12))) @ V.T
        s = np.linalg.svd(Bh @ F @ Bh, compute_uv=False)
    else:
        s = np.linalg.svd(F, compute_uv=False)
    nrm = np.sqrt(np.sum(s * s))
    for r in range(4, 16):
        if r >= len(s) or np.sqrt(np.sum(s[r:] ** 2)) <= 4e-3 * nrm:
            return r
    return 16


def _make_in_maps(x, filt, rank, consts=None):
    if consts is None:
        consts = _make_consts(filt, rank)
    np_dt = mybir.dt.np(DT_MM)
    imgs = x.reshape(N_CORES, N_IMG, H, H)
    base = {
        "uyt": consts["uyt"].astype(np_dt), "uxt": consts["uxt"].astype(np_dt),
        "nt": consts["nt"].astype(np_dt), "mt": consts["mt"].astype(np_dt),
    }
    if consts["nr8"]:
        base["nt8"] = consts["nt8"]
        base["mt8"] = consts["mt8"]
    return [{"x": np.ascontiguousarray(imgs[core]).astype(np_dt), **base}
            for core in range(N_CORES)]


_RUNNER_CACHE = {}


def _get_runner(nc):
    """Persistent jitted 8-core runner (mirrors bass2jax.run_bass_via_pjrt's
    multi-core path) so repeated kernel() calls reuse one compiled executable."""
    if id(nc) in _RUNNER_CACHE:
        return _RUNNER_CACHE[id(nc)]
    import jax
    from jax.sharding import Mesh, PartitionSpec
    from jax.experimental.shard_map import shard_map
    from concourse.bass2jax import (_bass_exec_p, install_neuronx_cc_hook,
                                    partition_id_tensor)
    install_neuronx_cc_hook()
    in_names, out_names, out_avals, zero_outs = [], [], [], []
    for alloc in nc.m.functions[0].allocations:
        if not isinstance(alloc, mybir.MemoryLocationSet):
            continue
        name = alloc.memorylocations[0].name
        if alloc.kind == "ExternalInput":
            if nc.partition_id_tensor is not None and name == nc.partition_id_tensor.name:
                continue
            in_names.append(name)
        elif alloc.kind == "ExternalOutput":
            out_names.append(name)
            shape = tuple(alloc.tensor_shape)
            dtype = mybir.dt.np(alloc.dtype)
            out_avals.append(jax.core.ShapedArray(shape, dtype))
            zero_outs.append(np.zeros(shape, dtype))
    n_params = len(in_names)
    all_in_names = in_names + out_names
    if nc.partition_id_tensor is not None:
        all_in_names = all_in_names + [nc.partition_id_tensor.name]

    def _body(*args):
        operands = list(args)
        if nc.partition_id_tensor is not None:
            operands.append(partition_id_tensor())
        return tuple(_bass_exec_p.bind(
            *operands,
            out_avals=tuple(out_avals),
            in_names=tuple(all_in_names),
            out_names=tuple(out_names),
            lowering_input_output_aliases=(),
            sim_require_finite=True,
            sim_require_nnan=True,
            nc=nc,
        ))

    donate = tuple(range(n_params, n_params + len(out_names)))
    devices = jax.devices()[:N_CORES]
    mesh = Mesh(np.asarray(devices), ("core",))
    in_specs = (PartitionSpec("core"),) * (n_params + len(out_names))
    out_specs = (PartitionSpec("core"),) * len(out_names)
    sharded = jax.jit(
        shard_map(_body, mesh=mesh, in_specs=in_specs, out_specs=out_specs,
                  check_rep=False),
        donate_argnums=donate, keep_unused=True)
    runner = (sharded, in_names, out_names, out_avals, zero_outs)
    _RUNNER_CACHE[id(nc)] = runner
    return runner


def run(x, filt):
    """Run on 8 cores. Returns out [B,C,H,W] f32."""
    x = np.ascontiguousarray(np.asarray(x, dtype=np.float32))
    filt = np.asarray(filt, dtype=np.float32)
    B, C, Hh, Ww = x.shape
    assert (Hh, Ww) == (H, H) and B * C == N_CORES * N_IMG
    rank = _pick_rank(filt)
    consts = _make_consts(filt, rank)
    in_maps = _make_in_maps(x, filt, rank, consts)
    nc = _build_nc(rank=rank, k16=consts["k16"], nr8=consts["nr8"],
                   s_a=consts["s_a"], s_e=consts["s_e"])
    try:
        sharded, in_names, out_names, out_avals, zero_outs = _get_runner(nc)
        concat_in = [np.concatenate([in_maps[c][nm] for c in range(N_CORES)], axis=0)
                     for nm in in_names]
        concat_zero = [np.zeros((N_CORES * z.shape[0], *z.shape[1:]), z.dtype)
                       for z in zero_outs]
        outs = sharded(*concat_in, *concat_zero)
        oi = out_names.index("out")
        out = np.asarray(outs[oi]).reshape(N_CORES, *out_avals[oi].shape)
    except Exception:
        res = run_bass_kernel_spmd(nc, in_maps, core_ids=list(range(N_CORES)))
        out = np.stack([res.results[c]["out"] for c in range(N_CORES)])
    return out.reshape(B, C, H, H).astype(np.float32, copy=False)


def kernel(x, filt):
    return run(x, filt)



# revision 56
# speedup vs baseline: 1.3761x; 1.3761x over previous
"""AliasFreeActivation (upsample2x -> leaky_relu -> 31x31 depthwise sinc conv
-> downsample2x) as a Trainium2 Bass/Tile kernel, data-parallel over 8 cores.

Math (per [128,128] image; B*C = 512 images, 64 per core):
  out = Dy @ Conv_F(lrelu(Uy @ x @ Ux^T)) @ Dx^T
With F = sum_r g_r h_r^T (SVD of the 31x31 filter, effective rank 11):
  out = sum_r M_r @ act @ N_r^T
  M_r = Dy @ Toeplitz(g_r) [128,256],  N_r = Dx @ Toeplitz(h_r) [128,256]
  act = lrelu(Uy @ x @ Ux^T) [256,256]
All heavy work is dense matmuls on TensorE; downsample is folded into M/N.

Device dataflow per image (out[m,n] = sum_k lhsT[k,m] rhs[k,n]):
  S1a: tmpT[c,Y]    = sum_y x[y,c] UyT[y,Y]
  S1b: actT[X,Y]    = sum_c UxT[c,X] tmpT[c,Y]   (+ lrelu on evac)
  A:   W[Y,(r,j)]   = sum_X actT[X,Y] NT[X,(r,j)]
  B:   out[i,(m,j)] = sum_{r,Yc} MT[Yc,(r,i)] W[Yc,(r,m,j)]   (PSUM accum)
"""
import contextlib
import os

import numpy as np

import concourse.bass as bass
import concourse.mybir as mybir
import concourse.tile as tile
from concourse import bacc
from concourse.bass_utils import run_bass_kernel_spmd

H = 128
H2 = 256
KF = 31
LRELU_SLOPE = 0.01
RANK_ENV = os.environ.get("AFA_RANK")
RANK = int(RANK_ENV) if RANK_ENV else 11   # default; run() adapts to the filter
BAND_A = os.environ.get("AFA_BAND", "1") == "1"
GROUP = 4
# PSUM->SBUF evacuation engine assignment (v=DVE, s=ScalarE/ACT, p=Pool):
# [0] fp16-W evac, [1] fp8-W evac, [4] tmpT, [5] out-copy.
# NOTE: GPSIMD/Pool cannot read PSUM (verifier-enforced), so only v/s here.
def _evac():
    return os.environ.get("AFA_EVAC", "vsvs_v_s").replace("_", "")
N_CORES = 8
N_IMG = 64                      # images per core (512 / 8)
DT_MM = {
    "float32": mybir.dt.float32,
    "float32r": mybir.dt.float32r,
    "bfloat16": mybir.dt.bfloat16,
    "float16": mybir.dt.float16,
}[os.environ.get("AFA_DT", "float16")]  # matmul operand dtype


# ---------------- host-side constants ----------------

def _ac_matrix(out_n, in_n):
    scale = (in_n - 1) / (out_n - 1)
    c = np.arange(out_n, dtype=np.float64) * scale
    i0 = np.clip(np.floor(c).astype(np.int64), 0, in_n - 2)
    w = c - i0
    M = np.zeros((out_n, in_n), dtype=np.float64)
    M[np.arange(out_n), i0] = 1.0 - w
    M[np.arange(out_n), i0 + 1] = w
    return M


def _toeplitz_same(h, n):
    T = np.zeros((n, n), dtype=np.float64)
    for u in range(len(h)):
        d = u - len(h) // 2
        if d >= 0:
            idx = np.arange(0, n - d)
        else:
            idx = np.arange(-d, n)
        T[idx, idx + d] += h[u]
    return T


def _segs_of(rank):
    """Split the rank-stacked 128-col blocks into PSUM-bank segments <= 512."""
    segs = []
    r = 0
    while r < rank:
        nr = min(4, rank - r)
        segs.append((r, nr))
        r += nr
    return segs


def _shift_mat(n, d):
    S = np.zeros((n, n))
    idx = np.arange(0, n - d) if d >= 0 else np.arange(-d, n)
    S[idx, idx + d] = 1.0
    return S


def _weighted_rank(F, rank):
    """Rank-`rank` approx of F minimizing the end-to-end error under the
    signal model act ~ U x U^T with white x: err = ||B^1/2 (F-Fr) B^1/2||_F
    with B the Gram of the composed per-tap maps Z_u = D S_u U."""
    kf = F.shape[0]
    D = _ac_matrix(H, H2)
    Uu = _ac_matrix(H2, H)
    Zs = [D @ _shift_mat(H2, u - kf // 2) @ Uu for u in range(kf)]
    B = np.zeros((kf, kf))
    for u in range(kf):
        for v in range(u, kf):
            B[u, v] = B[v, u] = np.sum(Zs[u] * Zs[v])
    w, V = np.linalg.eigh(B)
    w = np.maximum(w, 1e-12)
    Bh = (V * np.sqrt(w)) @ V.T
    Bih = (V / np.sqrt(w)) @ V.T
    Gm = Bh @ F @ Bh
    U_, S_, Vt_ = np.linalg.svd(Gm)
    Fr = Bih @ (U_[:, :rank] * S_[:rank]) @ Vt_[:rank] @ Bih
    return Fr


FP8_ON = os.environ.get("AFA_FP8", "1") == "1"


def _sample_act(Uu, n=4):
    rng = np.random.default_rng(1234)
    X = rng.standard_normal((n, H, H))
    A = Uu @ X @ Uu.T
    return np.where(A >= 0, A, LRELU_SLOPE * A)


def _make_consts(filt, rank, seg_layout=True):
    """Build device constants. Ranks [0, k16) go to fp16 nt/mt in seg layout
    (nt columns (seg, j, r_local)-major for pass-A banded 2D APs; mt
    rank-major). When fp8 is active, the nr8 tail ranks go to nt8/mt8 in
    DoubleRow-interleaved fp8 layout, with per-rank scale balancing so act8,
    nt8, wg8 and mt8 all sit inside the fp8e4m3 normal range."""
    F = np.asarray(filt, dtype=np.float64)
    if os.environ.get("AFA_WSVD", "1") == "1" and rank < min(F.shape):
        F = _weighted_rank(F, rank)
    U, S, Vt = np.linalg.svd(F)
    D = _ac_matrix(H, H2)
    Uu = _ac_matrix(H2, H)
    uyt = np.ascontiguousarray(Uu.T).astype(np.float32)     # [128 y, 256 Y]

    nr8 = rank - 4 if (FP8_ON and seg_layout and 4 < rank <= 8) else 0
    k16 = rank - nr8

    Ms = [D @ _toeplitz_same(U[:, r] * np.sqrt(S[r]), H2) for r in range(rank)]
    Ns = [D @ _toeplitz_same(Vt[r, :] * np.sqrt(S[r]), H2) for r in range(rank)]

    nt = np.zeros((2, H, k16 * H), dtype=np.float32)
    mt = np.zeros((2, H, k16 * H), dtype=np.float32)
    segs = _segs_of(k16)
    for r in range(k16):
        if seg_layout:
            off = 0
            for (rs, nr) in segs:
                if rs <= r < rs + nr:
                    cols = off + np.arange(H) * nr + (r - rs)
                    break
                off += nr * H
        else:
            cols = np.arange(r * H, (r + 1) * H)
        for c in range(2):
            nt[c, :, cols] = Ns[r][:, c * H:(c + 1) * H].astype(np.float32)
            mt[c, :, r * H:(r + 1) * H] = Ms[r][:, c * H:(c + 1) * H].T.astype(np.float32)
    out = {"uyt": uyt, "uxt": uyt.copy(), "nt": nt, "mt": mt,
           "k16": k16, "nr8": nr8, "s_a": 1.0, "s_e": 1.0}
    if nr8 == 0:
        return out

    # ---- fp8 tail: per-rank scales ----
    act = _sample_act(Uu)
    actmax = np.abs(act).max() * 1.15
    s_a = 100.0 / actmax
    wmaxs = [np.abs(act @ Ns[r].T).max() * 1.3 for r in range(k16, rank)]
    mmaxs = [np.abs(Ms[r]).max() for r in range(k16, rank)]
    nmaxs = [np.abs(Ns[r]).max() for r in range(k16, rank)]
    # wg8_max_r * mt8_max_r = wmax_r * mmax_r is scale-invariant; split it
    # sqrt-balanced (bias wg8 up 2x), then choose global s_e to center nt8.
    bias = 2.0
    tmp = [np.sqrt(wmaxs[i] * mmaxs[i]) * bias / (s_a * wmaxs[i])
           for i in range(nr8)]           # = n_r * s_e per rank
    s_e = float(np.exp(np.mean(np.log([tmp[i] * nmaxs[i] for i in range(nr8)]))) / 100.0)
    n_rs = [tmp[i] / s_e for i in range(nr8)]
    m_rs = [1.0 / (s_a * n_rs[i] * s_e) for i in range(nr8)]

    np8 = mybir.dt.np(mybir.dt.float8e4)
    nt8 = np.zeros((H, 2, nr8 * H), dtype=np.float64)
    mt8 = np.zeros((H, 2, nr8 * H), dtype=np.float64)
    for i in range(nr8):
        r = k16 + i
        for c in range(2):
            # nt8[p, xc, j*nr8+i] = n_r * N_r[j, xc*128+p]
            nt8[:, c, i::nr8] = (n_rs[i] * Ns[r][:, c * H:(c + 1) * H]).T
            # mt8[p, yc, i*H+k] = m_r * M_r[k, yc*128+p]
            mt8[:, c, i * H:(i + 1) * H] = (m_rs[i] * Ms[r][:, c * H:(c + 1) * H]).T
    out["nt8"] = nt8.astype(np8).reshape(H, 2 * nr8 * H)
    out["mt8"] = mt8.astype(np8).reshape(H, 2 * nr8 * H)
    out["s_a"] = float(s_a)
    out["s_e"] = float(s_e)
    if os.environ.get("AFA_SAFOLD", "1") == "1":
        # Fold s_a into uyt so act comes out of S1/lrelu pre-scaled (lrelu is
        # positively homogeneous) and act8 becomes a PLAIN fp16->fp8 copy;
        # nt absorbs 1/s_a so the fp16-path W is unchanged. nt8 already
        # expects s_a-scaled act.
        out["uyt"] = (out["uyt"] * s_a).astype(np.float32)
        out["nt"] = (out["nt"] / s_a).astype(np.float32)
        out["s_a"] = 1.0
    return out


# ---------------- device program ----------------

def _build_tile_program(tc, outs, ins, *, n_img, rank, group, dt_mm, loop_reps=1,
                        k16=None, nr8=0, s_a=1.0, s_e=1.0):
    nc = tc.nc
    if nr8:
        x_d, uyt_d, uxt_d, nt_d, mt_d, nt8_d, mt8_d = ins
    else:
        x_d, uyt_d, uxt_d, nt_d, mt_d = ins
    out_d = outs[0]
    if k16 is None:
        k16 = rank
    RC = k16 * H
    G = group
    GW = G * H
    assert n_img % G == 0
    f32 = mybir.dt.float32
    fp8 = mybir.dt.float8e4
    DR = mybir.MatmulPerfMode.DoubleRow

    EVAC = _evac()
    segs = _segs_of(k16)
    n_units = 2 * k16 + nr8      # pass-B matmul count per group

    def _copy(eng, dst, src, scale=None):
        if eng == "v":
            if scale is None:
                nc.vector.tensor_copy(dst, src)
            else:
                nc.vector.tensor_scalar_mul(dst, src, scale)
        elif eng == "s":
            if scale is None:
                nc.scalar.activation(dst, src, mybir.ActivationFunctionType.Copy)
            else:
                nc.scalar.activation(dst, src, mybir.ActivationFunctionType.Copy,
                                     scale=scale)
        else:
            nc.gpsimd.tensor_copy(dst, src)

    ctx = contextlib.ExitStack()
    with ctx:
        const_pool = ctx.enter_context(tc.tile_pool(name="consts", bufs=1))
        x_pool = ctx.enter_context(tc.tile_pool(name="x", bufs=2))
        tmp_pool = ctx.enter_context(tc.tile_pool(
            name="tmp", bufs=int(os.environ.get("AFA_TMPB", "3"))))
        act_pool = ctx.enter_context(tc.tile_pool(
            name="act", bufs=int(os.environ.get("AFA_ACTB", "6"))))
        w_pool = ctx.enter_context(tc.tile_pool(
            name="w", bufs=int(os.environ.get("AFA_WB", "2"))))
        osb_pool = ctx.enter_context(tc.tile_pool(name="osb", bufs=2))
        # shared small-PSUM pool (tmpT pairs + act tiles), one bank per buf
        ps_s = ctx.enter_context(tc.tile_pool(
            name="ps_s", bufs=int(os.environ.get("AFA_PSS", "3")), space="PSUM"))
        ps_w = ctx.enter_context(tc.tile_pool(
            name="ps_w", bufs=int(os.environ.get("AFA_PSW", "2")), space="PSUM"))
        ps_out = ctx.enter_context(tc.tile_pool(name="ps_out", bufs=1, space="PSUM"))

        uyt_sb = const_pool.tile([H, H2], dt_mm, tag="uyt")
        nc.sync.dma_start(uyt_sb[:], uyt_d[:])
        uxt_sb = const_pool.tile([H, H2], dt_mm, tag="uxt")
        nc.sync.dma_start(uxt_sb[:], uxt_d[:])
        nt_sb = []
        mt_sb = []
        for c in range(2):
            t = const_pool.tile([H, RC], dt_mm, tag=f"nt{c}", name=f"nt{c}_sb")
            nc.sync.dma_start(t[:], nt_d[c])
            nt_sb.append(t)
            t = const_pool.tile([H, RC], dt_mm, tag=f"mt{c}", name=f"mt{c}_sb")
            nc.sync.dma_start(t[:], mt_d[c])
            mt_sb.append(t)
        if nr8:
            nt8_sb = const_pool.tile([H, 2 * nr8 * H], fp8, tag="nt8")
            nc.sync.dma_start(nt8_sb[:], nt8_d[:])
            mt8_sb = const_pool.tile([H, 2 * nr8 * H], fp8, tag="mt8")
            nc.sync.dma_start(mt8_sb[:], mt8_d[:])

        def _fetch_x(g):
            x_sb = x_pool.tile([H, GW], dt_mm, tag="x", name=f"x_{g}")
            xg = x_d[g * G:(g + 1) * G].rearrange("g h w -> h g w")
            nc.sync.dma_start(x_sb[:].rearrange("h (g w) -> h g w", g=G), xg)
            return x_sb

        def _emit_all_groups():
            ngroups = n_img // G
            pending = None
            x_next = _fetch_x(0)
            for g in range(ngroups):
                x_cur = x_next
                x_next = _fetch_x(g + 1) if g + 1 < ngroups else None
                pending = _emit_group(g, pending, x_cur)
            if pending is not None:
                nch = 4 - int(os.environ.get("AFA_PBD", "1"))
                for ci in range(nch):
                    pending(ci, chunks=nch)

        def _emit_group(g, pending_b, x_sb):

            wg_sb = w_pool.tile([H, 2 * k16 * GW], dt_mm, tag="wg",
                                name=f"wg_{g}")
            wgv = wg_sb[:].rearrange("p (c r g w) -> p c r g w", c=2, r=k16, g=G)
            wg8_sb = None
            if nr8:
                wg8_sb = w_pool.tile([H, 2 * nr8 * GW], fp8, tag="wg8",
                                     name=f"wg8_{g}")

            # phases 1+2 interleaved by half so ps_s slots for S1a pairs are
            # recycled from fast tmpT copies, not slow lrelus: the alloc
            # sequence tp0,ap0,ap1,tp1,ap2,ap3 with 3 bufs makes each tp wait
            # on the previous tp's DVE copy instead of an ACT lrelu.
            act_sbs = []
            act8_sbs = []
            for half in range(G // 2):
                tmpT_ps = ps_s.tile([H, 2 * H2], f32, tag="sp",
                                    name=f"tp_{g}_{half}")
                for u in range(2):
                    m = half * 2 + u
                    nc.tensor.matmul(tmpT_ps[:, u * H2:(u + 1) * H2],
                                     x_sb[:, m * H:(m + 1) * H], uyt_sb[:],
                                     start=True, stop=True)
                t_sb = tmp_pool.tile([H, 2 * H2], dt_mm, tag="tmpT")
                _copy(EVAC[4], t_sb[:], tmpT_ps[:])
                for u in range(2):
                    m = half * 2 + u
                    act_ps = ps_s.tile([H, 2 * H2], f32, tag="sp",
                                       name=f"ap_{g}_{m}")
                    tw = t_sb[:, u * H2:(u + 1) * H2]
                    for xc in range(2):
                        nc.tensor.matmul(act_ps[:, xc * H2:(xc + 1) * H2],
                                         uxt_sb[:, xc * H:(xc + 1) * H], tw,
                                         start=True, stop=True)
                    act_sb = act_pool.tile([H, 2 * H2], dt_mm, tag="act")
                    nc.scalar.activation(act_sb[:], act_ps[:],
                                         mybir.ActivationFunctionType.Lrelu,
                                         alpha=LRELU_SLOPE)
                    act_sbs.append(act_sb)
                    if nr8:
                        act8_sb = act_pool.tile([H, 2 * H2], fp8, tag="act8")
                        a8e = os.environ.get("AFA_ACT8ENG", "v")
                        if s_a == 1.0:
                            _copy(a8e, act8_sb[:], act_sb[:])
                        elif a8e == "p":
                            nc.gpsimd.tensor_scalar_mul(act8_sb[:], act_sb[:], s_a)
                        elif a8e == "v":
                            nc.vector.tensor_scalar_mul(act8_sb[:], act_sb[:], s_a)
                        else:
                            nc.scalar.activation(act8_sb[:], act_sb[:],
                                                 mybir.ActivationFunctionType.Copy,
                                                 scale=s_a)
                        act8_sbs.append(act8_sb)

            # phase 3: pass A per image, with the previous group's pass-B
            # matmuls interleaved between images so the evac engines always
            # have PE work to hide behind.
            # fp16 part: nt/W_ps seg columns are (j, r_local)-major, so the
            # Toeplitz j-band of each X-chunk is a CONTIGUOUS column window:
            # X-chunk0 only reaches j<=71, chunk1 only j>=56; j in [56,72)
            # accumulates (has_written set by mm1), the rest first-write.
            # fp8 part: one DoubleRow matmul per yc contracts both X-chunks.
            pbd = int(os.environ.get("AFA_PBD", "1"))   # passB start image
            for m in range(G):
                if pending_b is not None and m >= pbd:
                    pending_b(m - pbd, chunks=G - pbd)
                act_sb = act_sbs[m]
                soff = 0
                for si, (rs, nr) in enumerate(segs):
                    sw = nr * H
                    w_ps = ps_w.tile([H, 1024], f32, tag="wps",
                                     name=f"wps_{g}_{m}_{si}")
                    jwin = ((0, 72), (56, H)) if BAND_A else ((0, H), (0, H))
                    for yc in range(2):
                        for xc in range(2):
                            j0, j1 = jwin[xc]
                            nc.tensor.matmul(
                                w_ps[:, yc * 512 + j0 * nr:yc * 512 + j1 * nr],
                                act_sb[:, xc * H2 + yc * H: xc * H2 + (yc + 1) * H],
                                nt_sb[xc][:, soff + j0 * nr:soff + j1 * nr],
                                start=(xc == 0), stop=(xc == 1),
                                skip_group_check=BAND_A)
                    # evac: seg cols (yc-bank, j, r_local) -> wg (yc, r, m, j)
                    eng = EVAC[(0 if nr8 else si % 2 * 2) % 4]
                    if sw == 512:
                        src = w_ps[:].rearrange("p (c j r) -> p c r j", c=2, r=nr)
                        _copy(eng, wgv[:, :, rs:rs + nr, m], src)
                    else:
                        for yc in range(2):
                            src = w_ps[:, yc * 512:yc * 512 + sw].rearrange(
                                "p (j r) -> p r j", r=nr)
                            _copy(eng, wgv[:, yc, rs:rs + nr, m], src)
                    soff += sw
                if nr8:
                    act8 = act8_sbs[m][:].rearrange("p (x y) -> p x y", x=2)
                    nt8v = nt8_sb[:].rearrange("p (x c) -> p x c", x=2)
                    w_ps8 = ps_w.tile([H, 1024], f32, tag="wps",
                                      name=f"wps8_{g}_{m}")
                    for yc in range(2):
                        nc.tensor.matmul(
                            w_ps8[:, yc * 512:(yc + 1) * 512],
                            act8[:, :, yc * H:(yc + 1) * H],
                            nt8v,
                            start=True, stop=True, perf_mode=DR)
                    src = w_ps8[:].rearrange("p (c j r) -> p c r j", c=2, r=nr8)
                    dst = wg8_sb[:].rearrange(
                        "p (c r g w) -> p c r g w", c=2, r=nr8, g=G)[:, :, :, m]
                    _copy(EVAC[1], dst, src, scale=s_e)

            # pass B emitted in `chunks` slices; slice ci==chunks-1 finishes
            # the accumulation, evacuates and DMAs out
            state = {"out_ps": None, "nmm": 0}
            units = [("16", yc, r) for yc in range(2) for r in range(k16)]
            units += [("8", 0, rl) for rl in range(nr8)]

            def _pass_b(ci, chunks=G):
                if state["out_ps"] is None:
                    state["out_ps"] = ps_out.tile([H, GW], f32, tag="ops",
                                                  name=f"ops_{g}")
                out_ps = state["out_ps"]
                n0 = (ci * n_units) // chunks
                n1 = ((ci + 1) * n_units) // chunks
                for kind, yc, r in units[n0:n1]:
                    state["nmm"] += 1
                    if kind == "16":
                        nc.tensor.matmul(
                            out_ps[:],
                            mt_sb[yc][:, r * H:(r + 1) * H],
                            wgv[:, yc, r],
                            start=(state["nmm"] == 1),
                            stop=(state["nmm"] == n_units),
                            skip_group_check=True)
                    else:
                        mt8v = mt8_sb[:].rearrange("p (c ri) -> p c ri", c=2)
                        wg8v = wg8_sb[:].rearrange(
                            "p (c r gw) -> p c r gw", c=2, r=nr8)
                        nc.tensor.matmul(
                            out_ps[:],
                            mt8v[:, :, r * H:(r + 1) * H],
                            wg8v[:, :, r],
                            start=(state["nmm"] == 1),
                            stop=(state["nmm"] == n_units),
                            perf_mode=DR, skip_group_check=True)
                if ci == chunks - 1:
                    og = out_d[g * G:(g + 1) * G].rearrange("g h w -> h g w")
                    out_sb = osb_pool.tile([H, GW], f32, tag="osb")
                    _copy(EVAC[5], out_sb[:], out_ps[:])
                    nc.sync.dma_start(
                        og, out_sb[:].rearrange("h (g w) -> h g w", g=G))

            return _pass_b

        if loop_reps > 1:
            with tc.For_i(0, loop_reps, 1):
                _emit_all_groups()
        else:
            _emit_all_groups()


_NC_CACHE = {}


def _build_nc(n_img=N_IMG, rank=RANK, group=GROUP, dt_mm=DT_MM, loop_reps=1,
              k16=None, nr8=0, s_a=1.0, s_e=1.0):
    if k16 is None:
        k16 = rank
    key = (n_img, rank, group, dt_mm, loop_reps, k16, nr8,
           round(s_a, 9), round(s_e, 12), _evac(),
           os.environ.get("AFA_ACT8ENG", "v"), os.environ.get("AFA_PSW", ""),
           os.environ.get("AFA_PSS", ""))
    if key in _NC_CACHE:
        return _NC_CACHE[key]
    nc = bacc.Bacc("TRN2", target_bir_lowering=False, debug=False)
    f32 = mybir.dt.float32
    fp8 = mybir.dt.float8e4
    x_d = nc.dram_tensor("x", [n_img, H, H], dt_mm, kind="ExternalInput").ap()
    uyt_d = nc.dram_tensor("uyt", [H, H2], dt_mm, kind="ExternalInput").ap()
    uxt_d = nc.dram_tensor("uxt", [H, H2], dt_mm, kind="ExternalInput").ap()
    nt_d = nc.dram_tensor("nt", [2, H, k16 * H], dt_mm, kind="ExternalInput").ap()
    mt_d = nc.dram_tensor("mt", [2, H, k16 * H], dt_mm, kind="ExternalInput").ap()
    ins = [x_d, uyt_d, uxt_d, nt_d, mt_d]
    if nr8:
        nt8_d = nc.dram_tensor("nt8", [H, 2 * nr8 * H], fp8,
                               kind="ExternalInput").ap()
        mt8_d = nc.dram_tensor("mt8", [H, 2 * nr8 * H], fp8,
                               kind="ExternalInput").ap()
        ins += [nt8_d, mt8_d]
    out_d = nc.dram_tensor("out", [n_img, H, H], f32, kind="ExternalOutput").ap()
    with tile.TileContext(nc) as tc:
        _build_tile_program(tc, [out_d], ins,
                            n_img=n_img, rank=rank, group=group, dt_mm=dt_mm,
                            loop_reps=loop_reps, k16=k16, nr8=nr8,
                            s_a=s_a, s_e=s_e)
    nc.compile()
    _NC_CACHE[key] = nc
    return nc


def _pick_rank(filt):
    """Smallest rank whose weighted-truncation error estimate fits the
    error budget (harness gate 2e-2; leave room for fp16/fp8 quantization).
    For the reference's sinc filter this lands on 8."""
    if RANK_ENV:
        return int(RANK_ENV)
    F = np.asarray(filt, np.float64)
    if os.environ.get("AFA_WSVD", "1") == "1":
        kf = F.shape[0]
        D = _ac_matrix(H, H2)
        Uu = _ac_matrix(H2, H)
        Zs = [D @ _shift_mat(H2, u - kf // 2) @ Uu for u in range(kf)]
        B = np.zeros((kf, kf))
        for u in range(kf):
            for v in range(u, kf):
                B[u, v] = B[v, u] = np.sum(Zs[u] * Zs[v])
        w, V = np.linalg.eigh(B)
        Bh = (V * np.sqrt(np.maximum(w, 1e-12))) @ V.T
        s = np.linalg.svd(Bh @ F @ Bh, compute_uv=False)
    else:
        s = np.linalg.svd(F, compute_uv=False)
    nrm = np.sqrt(np.sum(s * s))
    for r in range(4, 16):
        if r >= len(s) or np.sqrt(np.sum(s[r:] ** 2)) <= 4e-3 * nrm:
            return r
    return 16


def _make_in_maps(x, filt, rank, consts=None):
    if consts is None:
        consts = _make_consts(filt, rank)
    np_dt = mybir.dt.np(DT_MM)
    imgs = x.reshape(N_CORES, N_IMG, H, H)
    base = {
        "uyt": consts["uyt"].astype(np_dt), "uxt": consts["uxt"].astype(np_dt),
        "nt": consts["nt"].astype(np_dt), "mt": consts["mt"].astype(np_dt),
    }
    if consts["nr8"]:
        base["nt8"] = consts["nt8"]
        base["mt8"] = consts["mt8"]
    return [{"x": np.ascontiguousarray(imgs[core]).astype(np_dt), **base}
            for core in range(N_CORES)]


_RUNNER_CACHE = {}


def _get_runner(nc):
    """Persistent jitted 8-core runner (mirrors bass2jax.run_bass_via_pjrt's
    multi-core path) so repeated kernel() calls reuse one compiled executable."""
    if id(nc) in _RUNNER_CACHE:
        return _RUNNER_CACHE[id(nc)]
    import jax
    from jax.sharding import Mesh, PartitionSpec
    from jax.experimental.shard_map import shard_map
    from concourse.bass2jax import (_bass_exec_p, install_neuronx_cc_hook,
                                    partition_id_tensor)
    install_neuronx_cc_hook()
    in_names, out_names, out_avals, zero_outs = [], [], [], []
    for alloc in nc.m.functions[0].allocations:
        if not isinstance(alloc, mybir.MemoryLocationSet):
            continue
        name = alloc.memorylocations[0].name
        if alloc.kind == "ExternalInput":
            if nc.partition_id_tensor is not None and name == nc.partition_id_tensor.name:
                continue
            in_names.append(name)
        elif alloc.kind == "ExternalOutput":
            out_names.append(name)
            shape = tuple(alloc.tensor_shape)
            dtype = mybir.dt.np(alloc.dtype)
            out_avals.append(jax.core.ShapedArray(shape, dtype))
            zero_outs.append(np.zeros(shape, dtype))
    n_params = len(in_names)
    all_in_names = in_names + out_names
    if nc.partition_id_tensor is not None:
        all_in_names = all_in_names + [nc.partition_id_tensor.name]

    def _body(*args):
        operands = list(args)
        if nc.partition_id_tensor is not None:
            operands.append(partition_id_tensor())
        return tuple(_bass_exec_p.bind(
            *operands,
            out_avals=tuple(out_avals),
            in_names=tuple(all_in_names),
            out_names=tuple(out_names),
            lowering_input_output_aliases=(),
            sim_require_finite=True,
            sim_require_nnan=True,
            nc=nc,
        ))

    donate = tuple(range(n_params, n_params + len(out_names)))
    devices = jax.devices()[:N_CORES]
    mesh = Mesh(np.asarray(devices), ("core",))
    in_specs = (PartitionSpec("core"),) * (n_params + len(out_names))
    out_specs = (PartitionSpec("core"),) * len(out_names)
    sharded = jax.jit(
        shard_map(_body, mesh=mesh, in_specs=in_specs, out_specs=out_specs,
                  check_rep=False),
        donate_argnums=donate, keep_unused=True)
    runner = (sharded, in_names, out_names, out_avals, zero_outs)
    _RUNNER_CACHE[id(nc)] = runner
    return runner


def run(x, filt):
    """Run on 8 cores. Returns out [B,C,H,W] f32."""
    x = np.ascontiguousarray(np.asarray(x, dtype=np.float32))
    filt = np.asarray(filt, dtype=np.float32)
    B, C, Hh, Ww = x.shape
    assert (Hh, Ww) == (H, H) and B * C == N_CORES * N_IMG
    rank = _pick_rank(filt)
    consts = _make_consts(filt, rank)
    in_maps = _make_in_maps(x, filt, rank, consts)
    nc = _build_nc(rank=rank, k16=consts["k16"], nr8=consts["nr8"],
                   s_a=consts["s_a"], s_e=consts["s_e"])
    try:
        sharded, in_names, out_names, out_avals, zero_outs = _get_runner(nc)
        concat_in = [np.concatenate([in_maps[c][nm] for c in range(N_CORES)], axis=0)
                     for nm in in_names]
        concat_zero = [np.zeros((N_CORES * z.shape[0], *z.shape[1:]), z.dtype)
                       for z in zero_outs]
        outs = sharded(*concat_in, *concat_zero)
        oi = out_names.index("out")
        out = np.asarray(outs[oi]).reshape(N_CORES, *out_avals[oi].shape)
    except Exception:
        res = run_bass_kernel_spmd(nc, in_maps, core_ids=list(range(N_CORES)))
        out = np.stack([res.results[c]["out"] for c in range(N_CORES)])
    return out.reshape(B, C, H, H).astype(np.float32, copy=False)


def kernel(x, filt):
    return run(x, filt)

